# revision 1
# baseline (speedup 1.0000x reference)
"""Trainium2 Bass kernel for nn_BasicNet (CondConv 3-branch + BN + channel shuffle).

Reference computation:
  x [32, 256, 56, 56] split into 4 channel groups of 64:
    s0 passthrough,
    sq = BN(CondConv3x3(s1)), vr = BN(CondConv3x1(s2)), hz = BN(CondConv1x3(s3))
  out = channel_shuffle(concat([s0, sq, vr, hz]), groups=8)

Sharding: data-parallel over batch (4 samples per core on 8 cores); BN batch
stats (per-channel sum / sum-of-squares) are all-reduced across cores.

v3 design notes (from HW profile of v2):
  - conv in bf16: fp32r matmuls measured ~2-3 cyc/col; bf16 streams 1 cyc/col.
    Host ships zero-padded bf16 branch images; per-sample conv weights are
    aggregated on DVE in f32 and cast to bf16 on the final accumulate.
  - tap pairing: the input tile holds the image on partitions 0:64 and the
    image shifted by one column (sq,h) / one row (v) on partitions 64:128
    (single DMA double-reads DRAM with an overlapping AP). Pairs of taps then
    contract as one K=128 matmul; leftover taps run K=64 on the lower half.
    35 + att matmuls per sample instead of 108.
  - conv outputs stored bf16 (halves SBUF + 2x DVE bn_stats); BN stats are
    computed from the stored bf16 values so normalization is self-consistent.
  - one store DMA per unit with the channel shuffle folded into the dest AP;
    normalize alternates ACT/DVE into f32 bounce tiles.
  - AllReduce payload halved by pre-combining partition halves; collective
    triggered from the (idle) tensor engine.
"""

import sys

if '/opt/trn_rl_repo' not in sys.path:
    sys.path.insert(0, '/opt/trn_rl_repo')

import numpy as np
import ml_dtypes

import concourse.bass as bass
import concourse.bacc as bacc
import concourse.tile as tile
from concourse import mybir
from concourse import bass_utils

F32 = mybir.dt.float32
BF16 = mybir.dt.bfloat16

N_CORES = 8
NS = 4                   # samples per core
H = W = 56
HW = H * W               # 3136
C = 64                   # channels per branch (Cin == O == 64)
KEXP = 4                 # CondConv experts
ROWS_PER_TILE = 8
NT = ROWS_PER_TILE * W   # 448 free elements per matmul tile
N_TILES = H // ROWS_PER_TILE  # 7
M_TOTAL = 32 * HW        # BN stat count
EPS = 1e-5
ROW_SLACK = 64           # extra zero elements per channel row (>= max shift)

# branch geometry:
#  bi: (name, padded (ph,pw), shift, pairs [(tap_lo, tap_hi)], singles [tap])
#  taps are (dy, dx); shift = element offset of the upper partition half
BR = [
    ('sq', (58, 58), 1, [((dy, 0), (dy, 1)) for dy in range(3)],
     [(dy, 2) for dy in range(3)]),
    ('v', (58, 56), 56, [((0, 0), (1, 0))], [(2, 0)]),
    ('h', (56, 58), 1, [((0, 0), (0, 1))], [(0, 2)]),
]


def _build_nc():
    nc = bacc.Bacc('TRN2', target_bir_lowering=False, debug=False,
                   num_devices=N_CORES)

    x0 = nc.dram_tensor('x0', [NS, C, HW], F32, kind='ExternalInput').ap()
    xp = {}
    w_t = {}
    for bi, (bn, (ph, pw), shift, pairs, singles) in enumerate(BR):
        xp[bi] = nc.dram_tensor(f'xp_{bn}', [NS, C, ph * pw + ROW_SLACK], BF16,
                                kind='ExternalInput').ap()
        ncol = len(pairs) + len(singles)
        w_t[bi] = nc.dram_tensor(f'w_{bn}', [128, KEXP, ncol * C], F32,
                                 kind='ExternalInput').ap()
    att_w = nc.dram_tensor('att_w', [C, 3, KEXP], F32, kind='ExternalInput').ap()
    att_b = nc.dram_tensor('att_b', [KEXP, 3], F32, kind='ExternalInput').ap()
    gb = nc.dram_tensor('gb', [C, 2, 3], F32, kind='ExternalInput').ap()
    out = nc.dram_tensor('out', [NS, 4 * C, H, W], F32,
                         kind='ExternalOutput').ap()

    with tile.TileContext(nc) as tc:
        _emit(tc, x0, xp, w_t, att_w, att_b, gb, out)

    nc.compile()
    return nc


def _emit(tc, x0, xp, w_t, att_w, att_b, gb, out):
    nc = tc.nc
    from contextlib import ExitStack
    ctx = ExitStack()
    with ctx:
        persist = ctx.enter_context(tc.tile_pool(name='persist', bufs=1))
        aggp = ctx.enter_context(tc.tile_pool(name='aggp', bufs=3))
        smalls = ctx.enter_context(tc.tile_pool(name='smalls', bufs=4))
        bouncep = ctx.enter_context(tc.tile_pool(name='bouncep', bufs=3))
        pscrp = ctx.enter_context(tc.tile_pool(name='pscrp', bufs=2))
        psum_conv = ctx.enter_context(
            tc.tile_pool(name='psum_conv', bufs=4, space='PSUM'))
        psum_att = ctx.enter_context(
            tc.tile_pool(name='psum_att', bufs=2, space='PSUM'))
        dram = ctx.enter_context(tc.tile_pool(name='dram', bufs=1, space='DRAM'))

        # ---------- persistent SBUF state ----------
        # doubled (shifted) bf16 input image tiles, ping-pong per branch
        in_tiles = {}
        for bi, (bn, (ph, pw), shift, pairs, singles) in enumerate(BR):
            for pp in range(3):
                t = persist.tile([128, ph * pw], BF16, tag=f'in_{bi}_{pp}',
                                 name=f'in_{bi}_{pp}')
                in_tiles[(bi, pp)] = t

        # expert weights [128, k, ncol*64]; upper half of single columns is 0
        w_sb = {}
        for bi, (bn, _, _, pairs, singles) in enumerate(BR):
            ncol = len(pairs) + len(singles)
            t = persist.tile([128, KEXP, ncol * C], F32, tag=f'w_sb_{bi}',
                             name=f'w_sb_{bi}')
            nc.gpsimd.dma_start(out=t, in_=w_t[bi])
            w_sb[bi] = t

        att_w_sb = persist.tile([C, 3, KEXP], F32, tag='att_w_sb')
        nc.gpsimd.dma_start(out=att_w_sb, in_=att_w)
        att_b_sb = persist.tile([KEXP, 3], F32, tag='att_b_sb')
        nc.gpsimd.dma_start(out=att_b_sb, in_=att_b)
        gb_sb = persist.tile([C, 2, 3], F32, tag='gb_sb')
        nc.gpsimd.dma_start(out=gb_sb, in_=gb)

        # conv outputs (bf16): 6 tiles, two units each (lower/upper half)
        out_tiles = [persist.tile([128, HW], BF16, tag=f'out_{i}', name=f'out_{i}')
                     for i in range(6)]

        # per-otile bn_stats: [128(c, unit pair), 7(tile), 6]
        bnst = [persist.tile([128, N_TILES, 6], F32, tag=f'bnst_{i}',
                             name=f'bnst_{i}')
                for i in range(6)]

        ov = out.rearrange('n (c2 g) h w -> n g c2 (h w)', g=8)
        cc_in = dram.tile([3, 2, NS, C], F32)   # (branch, stat, sample, channel)
        cc_out = dram.tile([3, 2, NS, C], F32)

        # ---------- per (sample, branch) units ----------
        for s in range(NS):
            for bi, (bn, (ph, pw), shift, pairs, singles) in enumerate(BR):
                u = s * 3 + bi
                half = u % 2
                p0 = 64 * half
                otile = out_tiles[u // 2]
                npair = len(pairs)
                flat = ph * pw
                flat_s = flat + ROW_SLACK

                # two 2D DMAs fill the halves (upper reads DRAM at +shift)
                it = in_tiles[(bi, s % 3)]
                xps = xp[bi][s]          # [C, flat_s]
                nc.sync.dma_start(out=it[0:64, :], in_=xps[:, 0:flat])
                nc.sync.dma_start(out=it[64:128, :], in_=xps[:, shift:shift + flat])
                it3 = it.rearrange('c (r q) -> c r q', q=pw)

                # attention: pooled sums -> sigmoid(att_w @ mean + b)
                pooled = smalls.tile([C, 1], F32, tag='pooled')
                if u % 2 == 0:
                    nc.vector.tensor_reduce(out=pooled, in_=it[0:64, :],
                                            axis=mybir.AxisListType.X,
                                            op=mybir.AluOpType.add)
                else:
                    pscr = pscrp.tile([C, 3364], BF16, tag='pscr')
                    nc.scalar.activation(out=pscr[:, :flat], in_=it[0:64, :],
                                         func=mybir.ActivationFunctionType.Copy,
                                         accum_out=pooled)
                att_ps = psum_att.tile([KEXP, 1], F32, tag='att_ps')
                nc.tensor.matmul(att_ps, lhsT=att_w_sb[:, bi, :], rhs=pooled,
                                 start=True, stop=True)
                att_s = smalls.tile([KEXP, 1], F32, tag='att_s')
                nc.scalar.activation(out=att_s, in_=att_ps,
                                     func=mybir.ActivationFunctionType.Sigmoid,
                                     bias=att_b_sb[:, bi:bi + 1])
                att_f = smalls.tile([1, KEXP], F32, tag='att_f')
                nc.gpsimd.dma_start(out=att_f, in_=att_s)
                att_bc = smalls.tile([128, KEXP], F32, tag='att_bc')
                nc.gpsimd.partition_broadcast(att_bc, att_f)

                # aggregate per-sample conv weights: agg = sum_k att[k] * w[k]
                ncol = len(pairs) + len(singles)
                agg = aggp.tile([128, ncol * C], F32, tag='agg')
                nc.vector.tensor_scalar_mul(out=agg, in0=w_sb[bi][:, 0],
                                            scalar1=att_bc[:, 0:1])
                for k in range(1, KEXP - 1):
                    nc.vector.scalar_tensor_tensor(
                        out=agg, in0=w_sb[bi][:, k], scalar=att_bc[:, k:k + 1],
                        in1=agg, op0=mybir.AluOpType.mult, op1=mybir.AluOpType.add)
                agg_r = aggp.tile([128, ncol * C], BF16, tag='agg_r')
                nc.vector.scalar_tensor_tensor(
                    out=agg_r, in0=w_sb[bi][:, KEXP - 1],
                    scalar=att_bc[:, KEXP - 1:KEXP], in1=agg,
                    op0=mybir.AluOpType.mult, op1=mybir.AluOpType.add)

                # conv: per N-tile, pairs K=128 then singles K=64, PSUM 0:64
                for t in range(N_TILES):
                    pt = psum_conv.tile([64, NT], F32, tag='pt')
                    nmm = npair + len(singles)
                    mi = 0
                    for j, ((dy, dx), _hi) in enumerate(pairs):
                        r0 = ROWS_PER_TILE * t + dy
                        rhs = it3[:, r0:r0 + ROWS_PER_TILE, dx:dx + W]
                        nc.tensor.matmul(
                            pt, lhsT=agg_r[:, j * C:(j + 1) * C], rhs=rhs,
                            start=(mi == 0), stop=(mi == nmm - 1))
                        mi += 1
                    for j, (dy, dx) in enumerate(singles):
                        r0 = ROWS_PER_TILE * t + dy
                        rhs = it3[0:64, r0:r0 + ROWS_PER_TILE, dx:dx + W]
                        nc.tensor.matmul(
                            pt, lhsT=agg_r[0:64, (npair + j) * C:(npair + j + 1) * C],
                            rhs=rhs, start=(mi == 0), stop=(mi == nmm - 1))
                        mi += 1
                    # evacuate to bf16 (cross-partition for odd units)
                    nc.scalar.activation(
                        out=otile[p0:p0 + 64, t * NT:(t + 1) * NT], in_=pt,
                        func=mybir.ActivationFunctionType.Copy)
                if half == 1:
                    # both halves of this out tile are complete: paired stats
                    i = u // 2
                    for t in range(N_TILES):
                        nc.vector.bn_stats(
                            out=bnst[i][:, t, :],
                            in_=otile[:, t * NT:(t + 1) * NT])
                    # stage this tile's per-unit sums for the collective now
                    red_mv = smalls.tile([128, 2], F32, tag='red_mv')
                    nc.vector.bn_aggr(out=red_mv, in_=bnst[i])
                    red2 = smalls.tile([128, 2], F32, tag='red2')
                    nc.vector.tensor_scalar_mul(out=red2[:, 0:1],
                                                in0=red_mv[:, 0:1],
                                                scalar1=float(N_TILES * NT))
                    tmp = smalls.tile([128, 1], F32, tag='tmp_red')
                    nc.vector.tensor_tensor(out=tmp, in0=red_mv[:, 0:1],
                                            in1=red_mv[:, 0:1],
                                            op=mybir.AluOpType.mult)
                    nc.vector.tensor_tensor(out=tmp, in0=tmp,
                                            in1=red_mv[:, 1:2],
                                            op=mybir.AluOpType.add)
                    nc.vector.tensor_scalar_mul(out=red2[:, 1:2], in0=tmp,
                                                scalar1=float(N_TILES * NT))
                    for h in range(2):
                        uu = 2 * i + h
                        s_, bi_ = uu // 3, uu % 3
                        nc.gpsimd.dma_start(
                            out=cc_in[bi_][:, s_, :].rearrange('stat c -> c stat'),
                            in_=red2[64 * h:64 * h + 64, :])


        # ---------- BN stats all-reduce (per-unit sums, staged above) ------
        nc.gpsimd.collective_compute(
            'AllReduce', mybir.AluOpType.add,
            replica_groups=[list(range(N_CORES))],
            ins=[cc_in.opt()], outs=[cc_out.opt()])
        # s0 passthrough rides in the collective's shadow
        nc.sync.dma_start(out=ov[:, 0], in_=x0[:, 0:32])
        nc.sync.dma_start(out=ov[:, 1], in_=x0[:, 32:64])

        gs4 = persist.tile([C, 3, 2, NS], F32, tag='gs4')
        for bi_ in range(3):
            nc.gpsimd.dma_start(
                out=gs4[:, bi_], in_=cc_out[bi_].rearrange('stat s c -> c stat s'))
        gs = persist.tile([C, 3, 2], F32, tag='gs')
        nc.vector.tensor_reduce(out=gs, in_=gs4, axis=mybir.AxisListType.X,
                                op=mybir.AluOpType.add)
        # mean / E[x^2] -> scale/bias
        mv = persist.tile([C, 3, 2], F32, tag='mv')
        nc.vector.tensor_scalar_mul(out=mv, in0=gs, scalar1=1.0 / M_TOTAL)
        var = persist.tile([C, 3], F32, tag='var')
        nc.vector.tensor_tensor(out=var, in0=mv[:, :, 0], in1=mv[:, :, 0],
                                op=mybir.AluOpType.mult)
        nc.vector.tensor_tensor(out=var, in0=mv[:, :, 1], in1=var,
                                op=mybir.AluOpType.subtract)
        sd = persist.tile([C, 3], F32, tag='sd')
        epst = persist.tile([C, 1], F32, tag='epst')
        nc.vector.memset(epst, EPS)
        nc.scalar.activation(out=sd, in_=var,
                             func=mybir.ActivationFunctionType.Sqrt, bias=epst)
        nc.vector.reciprocal(out=sd, in_=sd)
        scale2 = persist.tile([128, 3], F32, tag='scale2')
        bias2 = persist.tile([128, 3], F32, tag='bias2')
        nc.vector.tensor_tensor(out=scale2[0:64], in0=gb_sb[:, 0], in1=sd,
                                op=mybir.AluOpType.mult)
        tmpb = persist.tile([C, 3], F32, tag='tmpb')
        nc.vector.tensor_tensor(out=tmpb, in0=mv[:, :, 0], in1=scale2[0:64],
                                op=mybir.AluOpType.mult)
        nc.vector.tensor_tensor(out=bias2[0:64], in0=gb_sb[:, 1], in1=tmpb,
                                op=mybir.AluOpType.subtract)
        nc.gpsimd.dma_start(out=scale2[64:128], in_=scale2[0:64])
        nc.gpsimd.dma_start(out=bias2[64:128], in_=bias2[0:64])

        # ---------- normalize (ACT/DVE alternating) + 2D stores ----
        for i in range(6):
            bounce = bouncep.tile([128, HW], F32, tag='bounce',
                                  name=f'bounce_{i}')
            otile = out_tiles[i]
            for half in range(2):
                u = 2 * i + half
                s, bi = u // 3, u % 3
                p0 = 64 * half
                oh = otile[p0:p0 + 64, :]
                bh = bounce[p0:p0 + 64, :]
                if u % 2 == 0:
                    nc.scalar.activation(out=bh, in_=oh,
                                         func=mybir.ActivationFunctionType.Identity,
                                         bias=bias2[p0:p0 + 64, bi:bi + 1],
                                         scale=scale2[p0:p0 + 64, bi:bi + 1])
                else:
                    nc.vector.tensor_scalar(
                        out=bh, in0=oh,
                        scalar1=scale2[p0:p0 + 64, bi:bi + 1],
                        scalar2=bias2[p0:p0 + 64, bi:bi + 1],
                        op0=mybir.AluOpType.mult, op1=mybir.AluOpType.add)
                g1 = 2 * (bi + 1)
                nc.sync.dma_start(out=ov[s, g1], in_=bounce[p0:p0 + 32, :])
                nc.sync.dma_start(out=ov[s, g1 + 1],
                                  in_=bounce[p0 + 32:p0 + 64, :])


_NC_CACHE = None


def _get_nc():
    global _NC_CACHE
    if _NC_CACHE is None:
        _NC_CACHE = _build_nc()
    return _NC_CACHE


def _host_weights(w, pairs, singles):
    """w [K, O, Cin, kh, kw] -> [K, 128, ncol*64] f32 paired-lhsT layout."""
    k, o, cin, kh, kw = w.shape
    npair, nsing = len(pairs), len(singles)
    ncol = npair + nsing
    wt = np.zeros((k, 128, ncol * C), np.float32)
    for j, ((dy0, dx0), (dy1, dx1)) in enumerate(pairs):
        wt[:, 0:64, j * C:(j + 1) * C] = w[:, :, :, dy0, dx0].transpose(0, 2, 1)
        wt[:, 64:128, j * C:(j + 1) * C] = w[:, :, :, dy1, dx1].transpose(0, 2, 1)
    for j, (dy, dx) in enumerate(singles):
        wt[:, 0:64, (npair + j) * C:(npair + j + 1) * C] = \
            w[:, :, :, dy, dx].transpose(0, 2, 1)
    return np.ascontiguousarray(wt.transpose(1, 0, 2))


def _prep_in_maps(inputs):
    x = np.ascontiguousarray(inputs['x'], dtype=np.float32)
    n_total = x.shape[0]
    pads = [(1, 1), (1, 0), (0, 1)]
    xpad = []
    for bi, (bn, (ph, pw), shift, pairs, singles) in enumerate(BR):
        ph_, pw_ = pads[bi]
        sl = x[:, C * (bi + 1):C * (bi + 2)]
        p = np.zeros((n_total, C, ph * pw + ROW_SLACK), ml_dtypes.bfloat16)
        img = p[:, :, :ph * pw].reshape(n_total, C, ph, pw)
        img[:, :, ph_:ph_ + H, pw_:pw_ + W] = sl.astype(ml_dtypes.bfloat16)
        xpad.append(np.ascontiguousarray(p))
    x0_full = np.ascontiguousarray(x[:, 0:C].reshape(n_total, C, HW))

    shared = {}
    names = [('sq', 'w_sq', 'att_w_sq', 'att_b_sq', 'g_sq', 'b_sq'),
             ('v', 'w_v', 'att_w_v', 'att_b_v', 'g_v', 'b_v'),
             ('h', 'w_h', 'att_w_h', 'att_b_h', 'g_h', 'b_h')]
    att_w_all = np.zeros((C, 3, KEXP), np.float32)
    att_b_all = np.zeros((KEXP, 3), np.float32)
    gb_all = np.zeros((C, 2, 3), np.float32)
    for bi, (bn, wk, awk, abk, gk, bk) in enumerate(names):
        w = np.asarray(inputs[wk], dtype=np.float32)
        shared[f'w_{bn}'] = _host_weights(w, BR[bi][3], BR[bi][4])
        att_w_all[:, bi, :] = np.asarray(inputs[awk], np.float32).T / float(HW)
        att_b_all[:, bi] = np.asarray(inputs[abk], np.float32)
        gb_all[:, 0, bi] = np.asarray(inputs[gk], np.float32)
        gb_all[:, 1, bi] = np.asarray(inputs[bk], np.float32)
    shared['att_w'] = att_w_all
    shared['att_b'] = att_b_all
    shared['gb'] = gb_all

    in_maps = []
    for ci in range(N_CORES):
        m = dict(shared)
        sl = slice(ci * NS, (ci + 1) * NS)
        m['x0'] = x0_full[sl]
        for bi, (bn, _, _, _, _) in enumerate(BR):
            m[f'xp_{bn}'] = xpad[bi][sl]
        in_maps.append(m)
    return in_maps


def run_raw(inputs, trace=False, **kwargs):
    """Build+run; returns (full_output, BassKernelResults)."""
    nc = _get_nc()
    in_maps = _prep_in_maps(inputs)
    res = bass_utils.run_bass_kernel_spmd(
        nc, in_maps, core_ids=list(range(N_CORES)), trace=trace, **kwargs)
    full = np.concatenate([res.results[i]['out'] for i in range(N_CORES)], axis=0)
    return full, res


def kernel(**inputs):
    full, _ = run_raw(inputs)
    return full



# revision 28
# speedup vs baseline: 1.2655x; 1.2655x over previous
"""Trainium2 Bass kernel for nn_BasicNet (CondConv 3-branch + BN + channel shuffle).

Reference computation:
  x [32, 256, 56, 56] split into 4 channel groups of 64:
    s0 passthrough,
    sq = BN(CondConv3x3(s1)), vr = BN(CondConv3x1(s2)), hz = BN(CondConv1x3(s3))
  out = channel_shuffle(concat([s0, sq, vr, hz]), groups=8)

Sharding: data-parallel over batch (4 samples per core on 8 cores); BN batch
stats (per-channel sum / sum-of-squares) are all-reduced across cores,
pipelined per branch.

v4 design (from HW profile of v3):
  - sample-pairing: each [128, *] tile holds sample A on partitions 0:64 and
    sample B on 64:128. Conv matmuls are K=64/M=64 diagonal PE tiles
    ((0,0) for A, (64,64) for B) that run concurrently in the array.
    Single-read bf16 image loads (no shifted double-read).
  - 3 per-branch AllReduces fired as soon as each branch's convs finish;
    collective latency + inter-core skew overlap later convs and stores.
  - normalize+store per branch right after its AR; one 802KB store DMA per
    (branch, pair) with the channel shuffle folded into the dest AP.
    Loads ride the Sync HWDGE ring, stores the Scalar ring.
  - attention: pooled sums on GpSimd (idle engine), then tiny PE matmuls
    (att logits; diag -> masked broadcast) instead of gpsimd DMA/broadcast.
  - partition-fold (stats) and partition-dup (scale/bias) via tiny constant
    matmuls (fold_mask / dup_mask).
"""

import sys

if '/opt/trn_rl_repo' not in sys.path:
    sys.path.insert(0, '/opt/trn_rl_repo')

import numpy as np
import ml_dtypes

import concourse.bass as bass
import concourse.bacc as bacc
import concourse.tile as tile
from concourse import mybir
from concourse import bass_utils

F32 = mybir.dt.float32
BF16 = mybir.dt.bfloat16

N_CORES = 8
NS = 4                   # samples per core
NPAIR = 2                # sample pairs per core
H = W = 56
HW = H * W               # 3136
C = 64                   # channels per branch (Cin == O == 64)
KEXP = 4                 # CondConv experts
ROWS_PER_TILE = 8
NT = ROWS_PER_TILE * W   # 448 free elements per matmul tile
N_TILES = H // ROWS_PER_TILE  # 7
M_TOTAL = 32 * HW        # BN stat count
EPS = 1e-5

# branch geometry: (name, (padded ph, pw), taps [(dy, dx)])
BR = [
    ('sq', (58, 58), [(dy, dx) for dy in range(3) for dx in range(3)]),
    ('v', (58, 56), [(dy, 0) for dy in range(3)]),
    ('h', (56, 58), [(0, dx) for dx in range(3)]),
]


def _build_nc():
    nc = bacc.Bacc('TRN2', target_bir_lowering=False, debug=False,
                   num_devices=N_CORES)

    x0 = nc.dram_tensor('x0', [NS, C, HW], F32, kind='ExternalInput').ap()
    xp = {}
    w_t = {}
    for bi, (bn, (ph, pw), taps) in enumerate(BR):
        xp[bi] = nc.dram_tensor(f'xp_{bn}', [NPAIR, 128, ph * pw], BF16,
                                kind='ExternalInput').ap()
        w_t[bi] = nc.dram_tensor(f'w_{bn}', [128, KEXP, len(taps) * C], F32,
                                 kind='ExternalInput').ap()
    att_w = nc.dram_tensor('att_w', [128, 3, KEXP], F32, kind='ExternalInput').ap()
    att_b = nc.dram_tensor('att_b', [128, 3], F32, kind='ExternalInput').ap()
    diag_mask = nc.dram_tensor('diag_mask', [128, KEXP], F32,
                               kind='ExternalInput').ap()
    bc_mask = nc.dram_tensor('bc_mask', [128, 128], F32,
                             kind='ExternalInput').ap()
    fold_mask = nc.dram_tensor('fold_mask', [128, C], F32,
                               kind='ExternalInput').ap()
    dup_mask = nc.dram_tensor('dup_mask', [C, 128], F32,
                              kind='ExternalInput').ap()
    gb = nc.dram_tensor('gb', [C, 2, 3], F32, kind='ExternalInput').ap()
    out = nc.dram_tensor('out', [NS, 4 * C, H, W], F32,
                         kind='ExternalOutput').ap()

    with tile.TileContext(nc) as tc:
        _emit(tc, x0, xp, w_t, att_w, att_b, diag_mask, bc_mask, fold_mask,
              dup_mask, gb, out)

    nc.compile()
    return nc


def _emit(tc, x0, xp, w_t, att_w, att_b, diag_mask, bc_mask, fold_mask,
          dup_mask, gb, out):
    nc = tc.nc
    from contextlib import ExitStack
    ctx = ExitStack()
    with ctx:
        persist = ctx.enter_context(tc.tile_pool(name='persist', bufs=1))
        smalls = ctx.enter_context(tc.tile_pool(name='smalls', bufs=4))
        aggp = ctx.enter_context(tc.tile_pool(name='aggp', bufs=2))
        bouncep = ctx.enter_context(tc.tile_pool(name='bouncep', bufs=3))
        psum_conv = ctx.enter_context(
            tc.tile_pool(name='psum_conv', bufs=5, space='PSUM'))
        psum_small = ctx.enter_context(
            tc.tile_pool(name='psum_small', bufs=3, space='PSUM'))
        dram = ctx.enter_context(tc.tile_pool(name='dram', bufs=1, space='DRAM'))

        # channel-shuffled output view: [n, g, c2, hw]; concat-ch = g*32+c2
        ov4 = out.rearrange('n (c2 g) h w -> n g c2 (h w)', g=8)

        # ---------- constants / weights ----------
        consts = {}
        for name, ap_, shape in [
                ('att_w', att_w, [128, 3, KEXP]), ('att_b', att_b, [128, 3]),
                ('diag_mask', diag_mask, [128, KEXP]),
                ('bc_mask', bc_mask, [128, 128]),
                ('fold_mask', fold_mask, [128, C]),
                ('dup_mask', dup_mask, [C, 128])]:
            t = persist.tile(shape, F32, tag=f'c_{name}', name=f'c_{name}')
            nc.sync.dma_start(out=t, in_=ap_)
            consts[name] = t
        gb_sb = persist.tile([C, 2, 3], F32, tag='gb_sb')
        nc.sync.dma_start(out=gb_sb, in_=gb)
        epst = persist.tile([C, 1], F32, tag='epst')
        nc.vector.memset(epst, EPS)

        w_sb = {}
        for bi, (bn, _, taps) in enumerate(BR):
            t = persist.tile([128, KEXP, len(taps) * C], F32, tag=f'w_{bi}',
                             name=f'w_sb_{bi}')
            nc.sync.dma_start(out=t, in_=w_t[bi])
            w_sb[bi] = t

        # ---------- image loads (all up front; persistent tiles) ----------
        in_tiles = {}
        for bi, (bn, (ph, pw), taps) in enumerate(BR):
            for p in range(NPAIR):
                t = persist.tile([128, ph * pw], BF16, tag=f'in_{bi}_{p}',
                                 name=f'in_{bi}_{p}')
                nc.sync.dma_start(out=t, in_=xp[bi][p])
                in_tiles[(bi, p)] = t

        # s0 passthrough: direct DRAM->DRAM, on the store (scalar) ring
        nc.scalar.dma_start(out=ov4[:, 0], in_=x0[:, 0:32])
        nc.scalar.dma_start(out=ov4[:, 1], in_=x0[:, 32:64])

        # conv outputs (bf16) per (branch, pair)
        otiles = {}
        for bi in range(3):
            for p in range(NPAIR):
                otiles[(bi, p)] = persist.tile(
                    [128, HW], BF16, tag=f'ot_{bi}_{p}', name=f'ot_{bi}_{p}')

        # per-branch staging
        bnst = {(bi, p): persist.tile([128, N_TILES, 6], F32,
                                      tag=f'bnst_{bi}_{p}',
                                      name=f'bnst_{bi}_{p}')
                for bi in range(3) for p in range(NPAIR)}
        red = {bi: persist.tile([128, NPAIR, 2], F32, tag=f'red_{bi}',
                                name=f'red_{bi}')
               for bi in range(3)}
        cc_in = {bi: dram.tile([C, 2], F32, name=f'cc_in_{bi}')
                 for bi in range(3)}
        cc_out = {bi: dram.tile([C, 2], F32, name=f'cc_out_{bi}')
                  for bi in range(3)}
        sbias = {}   # [128, 2] (scale, bias) per branch, both halves

        def conv_pair(bi, p):
            """CondConv for sample pair p of branch bi."""
            bn, (ph, pw), taps = BR[bi]
            it = in_tiles[(bi, p)]
            ot = otiles[(bi, p)]
            ntap = len(taps)

            # attention: pooled sums (ACT/DVE alternating), logits+sigmoid,
            # masked broadcast via tiny PE matmuls
            pooled = smalls.tile([128, 1], F32, tag='pooled')
            if (bi + p) % 2 == 0:
                pscr = aggp.tile([128, 3364], BF16, tag='pscr', name='pscr')
                nc.scalar.activation(out=pscr[:, 0:it.shape[-1]], in_=it,
                                     func=mybir.ActivationFunctionType.Copy,
                                     accum_out=pooled)
            else:
                nc.vector.tensor_reduce(out=pooled, in_=it,
                                        axis=mybir.AxisListType.X,
                                        op=mybir.AluOpType.add)
            att_ps = psum_small.tile([128, KEXP], F32, tag='sm', name='att_ps')
            nc.tensor.matmul(att_ps[0:KEXP, 0:1],
                             lhsT=consts['att_w'][0:C, bi, :],
                             rhs=pooled[0:C, :], start=True, stop=True)
            nc.tensor.matmul(att_ps[C:C + KEXP, 0:1],
                             lhsT=consts['att_w'][C:128, bi, :],
                             rhs=pooled[C:128, :], start=True, stop=True)
            att_s = smalls.tile([128, 1], F32, tag='att_s')
            nc.scalar.activation(out=att_s, in_=att_ps[:, 0:1],
                                 func=mybir.ActivationFunctionType.Sigmoid,
                                 bias=consts['att_b'][:, bi:bi + 1])
            diag = smalls.tile([128, KEXP], F32, tag='diag')
            nc.vector.tensor_scalar_mul(out=diag, in0=consts['diag_mask'],
                                        scalar1=att_s)
            bc_ps = psum_small.tile([128, KEXP], F32, tag='sm', name='bc_ps')
            nc.tensor.matmul(bc_ps[0:C, :], lhsT=consts['bc_mask'][0:KEXP, 0:C],
                             rhs=diag[0:KEXP, :], start=True, stop=True)
            nc.tensor.matmul(bc_ps[C:128, :],
                             lhsT=consts['bc_mask'][C:C + KEXP, 0:C],
                             rhs=diag[C:C + KEXP, :], start=True, stop=True)
            att_bc = smalls.tile([128, KEXP], F32, tag='att_bc')
            nc.scalar.activation(out=att_bc, in_=bc_ps,
                                 func=mybir.ActivationFunctionType.Copy)

            # aggregate per-pair conv weights (f32 chain, bf16 final)
            w4 = w_sb[bi]
            agg = aggp.tile([128, ntap * C], F32, tag=f'agg_{bi}')
            nc.vector.tensor_scalar_mul(out=agg, in0=w4[:, 0],
                                        scalar1=att_bc[:, 0:1])
            for k in range(1, KEXP - 1):
                nc.vector.scalar_tensor_tensor(
                    out=agg, in0=w4[:, k], scalar=att_bc[:, k:k + 1],
                    in1=agg, op0=mybir.AluOpType.mult, op1=mybir.AluOpType.add)
            agg_r = aggp.tile([128, ntap * C], BF16, tag=f'aggr_{bi}')
            nc.vector.scalar_tensor_tensor(
                out=agg_r, in0=w4[:, KEXP - 1],
                scalar=att_bc[:, KEXP - 1:KEXP], in1=agg,
                op0=mybir.AluOpType.mult, op1=mybir.AluOpType.add)

            it3 = it.rearrange('c (r q) -> c r q', q=pw)
            for t in range(N_TILES):
                pt = psum_conv.tile([128, NT], F32, tag='pt')
                r0 = ROWS_PER_TILE * t
                for j, (dy, dx) in enumerate(taps):
                    st, sp = (j == 0), (j == ntap - 1)
                    nc.tensor.matmul(
                        pt[0:C, :], lhsT=agg_r[0:C, j * C:(j + 1) * C],
                        rhs=it3[0:C, r0 + dy:r0 + dy + ROWS_PER_TILE,
                                dx:dx + W],
                        start=st, stop=sp, skip_group_check=True)
                    nc.tensor.matmul(
                        pt[C:128, :], lhsT=agg_r[C:128, j * C:(j + 1) * C],
                        rhs=it3[C:128, r0 + dy:r0 + dy + ROWS_PER_TILE,
                                dx:dx + W],
                        start=st, stop=sp, skip_group_check=True)
                # evacuate to bf16 (ACT); stats on DVE
                dst = ot[:, t * NT:(t + 1) * NT]
                nc.scalar.activation(out=dst, in_=pt,
                                     func=mybir.ActivationFunctionType.Copy)
                nc.vector.bn_stats(out=bnst[(bi, p)][:, t, :], in_=dst)

            # pair stats -> (sum, sumsq) per partition
            mv = smalls.tile([128, 2], F32, tag='mv')
            nc.vector.bn_aggr(out=mv, in_=bnst[(bi, p)])
            r = red[bi]
            nc.vector.tensor_scalar_mul(out=r[:, p, 0:1], in0=mv[:, 0:1],
                                        scalar1=float(HW))
            tmp = smalls.tile([128, 1], F32, tag='tmp_q')
            nc.vector.tensor_tensor(out=tmp, in0=mv[:, 0:1], in1=mv[:, 0:1],
                                    op=mybir.AluOpType.mult)
            nc.vector.tensor_tensor(out=tmp, in0=tmp, in1=mv[:, 1:2],
                                    op=mybir.AluOpType.add)
            nc.vector.tensor_scalar_mul(out=r[:, p, 1:2], in0=tmp,
                                        scalar1=float(HW))

        def branch_reduce(bi):
            """Fold partition halves + pairs, stage, all-reduce branch bi."""
            fold_ps = psum_small.tile([C, 2 * NPAIR], F32, tag='sm',
                                      name='fold_ps')
            nc.tensor.matmul(fold_ps, lhsT=consts['fold_mask'],
                             rhs=red[bi].rearrange('q p s -> q (p s)'),
                             start=True, stop=True)
            fold_sb = smalls.tile([C, 2 * NPAIR], F32, tag='fold_sb')
            nc.scalar.activation(out=fold_sb, in_=fold_ps,
                                 func=mybir.ActivationFunctionType.Copy)
            cc_sb = smalls.tile([C, 2], F32, tag='cc_sb')
            nc.vector.tensor_tensor(out=cc_sb, in0=fold_sb[:, 0:2],
                                    in1=fold_sb[:, 2:4],
                                    op=mybir.AluOpType.add)
            nc.gpsimd.dma_start(out=cc_in[bi], in_=cc_sb)
            nc.gpsimd.collective_compute(
                'AllReduce', mybir.AluOpType.add,
                replica_groups=[list(range(N_CORES))],
                ins=[cc_in[bi].opt()], outs=[cc_out[bi].opt()])

        def branch_norm_store(bi):
            """Post-AR: scale/bias, normalize + store both pairs of branch."""
            ar_sb = smalls.tile([C, 2], F32, tag='ar_sb')
            nc.gpsimd.dma_start(out=ar_sb, in_=cc_out[bi])
            mv = smalls.tile([C, 2], F32, tag='mv2')
            nc.vector.tensor_scalar_mul(out=mv, in0=ar_sb,
                                        scalar1=1.0 / M_TOTAL)
            var = smalls.tile([C, 1], F32, tag='var')
            nc.vector.tensor_tensor(out=var, in0=mv[:, 0:1], in1=mv[:, 0:1],
                                    op=mybir.AluOpType.mult)
            nc.vector.tensor_tensor(out=var, in0=mv[:, 1:2], in1=var,
                                    op=mybir.AluOpType.subtract)
            sd = smalls.tile([C, 1], F32, tag='sd')
            nc.scalar.activation(out=sd, in_=var,
                                 func=mybir.ActivationFunctionType.Sqrt,
                                 bias=epst)
            nc.vector.reciprocal(out=sd, in_=sd)
            sb2 = smalls.tile([C, 2], F32, tag='sb2')
            nc.vector.tensor_tensor(out=sb2[:, 0:1], in0=gb_sb[:, 0, bi:bi + 1],
                                    in1=sd, op=mybir.AluOpType.mult)
            tmpb = smalls.tile([C, 1], F32, tag='tmpb')
            nc.vector.tensor_tensor(out=tmpb, in0=mv[:, 0:1], in1=sb2[:, 0:1],
                                    op=mybir.AluOpType.mult)
            nc.vector.tensor_tensor(out=sb2[:, 1:2], in0=gb_sb[:, 1, bi:bi + 1],
                                    in1=tmpb, op=mybir.AluOpType.subtract)
            dup_ps = psum_small.tile([128, 2], F32, tag='sm', name='dup_ps')
            nc.tensor.matmul(dup_ps, lhsT=consts['dup_mask'], rhs=sb2,
                             start=True, stop=True)
            sb128 = persist.tile([128, 2], F32, tag=f'sb128_{bi}')
            nc.scalar.activation(out=sb128, in_=dup_ps,
                                 func=mybir.ActivationFunctionType.Copy)
            sbias[bi] = sb128

            g1 = 2 * (bi + 1)
            for p in range(NPAIR):
                bounce = bouncep.tile([128, HW], F32, tag='bounce',
                                      name=f'bounce_{bi}_{p}')
                ot = otiles[(bi, p)]
                if (bi + p) % 2 == 0:
                    nc.scalar.activation(
                        out=bounce, in_=ot,
                        func=mybir.ActivationFunctionType.Identity,
                        bias=sb128[:, 1:2], scale=sb128[:, 0:1])
                else:
                    nc.vector.tensor_scalar(
                        out=bounce, in0=ot, scalar1=sb128[:, 0:1],
                        scalar2=sb128[:, 1:2], op0=mybir.AluOpType.mult,
                        op1=mybir.AluOpType.add)
                for s_ in range(2):
                    nc.scalar.dma_start(
                        out=ov4[2 * p + s_, g1],
                        in_=bounce[C * s_:C * s_ + 32, :])
                    nc.scalar.dma_start(
                        out=ov4[2 * p + s_, g1 + 1],
                        in_=bounce[C * s_ + 32:C * s_ + C, :])

        def raw_store(bi):
            # debug path: store conv outputs unnormalized
            g1 = 2 * (bi + 1)
            for p in range(NPAIR):
                bounce = bouncep.tile([128, HW], F32, tag='bounce',
                                      name=f'bounce_{bi}_{p}')
                nc.scalar.activation(out=bounce, in_=otiles[(bi, p)],
                                     func=mybir.ActivationFunctionType.Copy)
                for s_ in range(2):
                    nc.scalar.dma_start(
                        out=ov4[2 * p + s_, g1],
                        in_=bounce[C * s_:C * s_ + 32, :])
                    nc.scalar.dma_start(
                        out=ov4[2 * p + s_, g1 + 1],
                        in_=bounce[C * s_ + 32:C * s_ + C, :])

        def conv_pair_noatt(bi, p):
            """Debug: conv with expert-0 weights, no attention chain."""
            bn, (ph, pw), taps = BR[bi]
            it = in_tiles[(bi, p)]
            ot = otiles[(bi, p)]
            ntap = len(taps)
            w4 = w_sb[bi]
            agg_r = aggp.tile([128, ntap * C], BF16, tag=f'aggr_{bi}',
                              name='agg_r')
            nc.vector.tensor_copy(out=agg_r, in_=w4[:, 0])
            it3 = it.rearrange('c (r q) -> c r q', q=pw)
            for t in range(N_TILES):
                pt = psum_conv.tile([128, NT], F32, tag='pt', name='pt')
                r0 = ROWS_PER_TILE * t
                for j, (dy, dx) in enumerate(taps):
                    st, sp = (j == 0), (j == ntap - 1)
                    nc.tensor.matmul(
                        pt[0:C, :], lhsT=agg_r[0:C, j * C:(j + 1) * C],
                        rhs=it3[0:C, r0 + dy:r0 + dy + ROWS_PER_TILE,
                                dx:dx + W],
                        start=st, stop=sp, skip_group_check=True)
                    nc.tensor.matmul(
                        pt[C:128, :], lhsT=agg_r[C:128, j * C:(j + 1) * C],
                        rhs=it3[C:128, r0 + dy:r0 + dy + ROWS_PER_TILE,
                                dx:dx + W],
                        start=st, stop=sp, skip_group_check=True)
                dst = ot[:, t * NT:(t + 1) * NT]
                nc.scalar.activation(out=dst, in_=pt,
                                     func=mybir.ActivationFunctionType.Copy)

        def conv_pair_part(bi, p, upto):
            """Debug: attention chain up to stage `upto`, then expert-0 conv."""
            bn, (ph, pw), taps = BR[bi]
            it = in_tiles[(bi, p)]
            ot = otiles[(bi, p)]
            ntap = len(taps)
            w4 = w_sb[bi]
            pooled = smalls.tile([128, 1], F32, tag='pooled', name='pooled')
            if (bi + p) % 2 == 0:
                pscr = aggp.tile([128, 3364], BF16, tag='pscr', name='pscr')
                nc.scalar.activation(out=pscr[:, 0:ph * pw], in_=it,
                                     func=mybir.ActivationFunctionType.Copy,
                                     accum_out=pooled)
            else:
                nc.vector.tensor_reduce(out=pooled, in_=it,
                                        axis=mybir.AxisListType.X,
                                        op=mybir.AluOpType.add)
            att_s = smalls.tile([128, 1], F32, tag='att_s', name='att_s')
            if upto >= 2:
                att_ps = psum_small.tile([128, KEXP], F32, tag='sm',
                                         name='att_ps')
                nc.tensor.matmul(att_ps[0:KEXP, 0:1],
                                 lhsT=consts['att_w'][0:C, bi, :],
                                 rhs=pooled[0:C, :], start=True, stop=True)
                nc.tensor.matmul(att_ps[C:C + KEXP, 0:1],
                                 lhsT=consts['att_w'][C:128, bi, :],
                                 rhs=pooled[C:128, :], start=True, stop=True)
                nc.scalar.activation(out=att_s, in_=att_ps[:, 0:1],
                                     func=mybir.ActivationFunctionType.Sigmoid,
                                     bias=consts['att_b'][:, bi:bi + 1])
            else:
                nc.vector.tensor_scalar_mul(out=att_s, in0=pooled,
                                            scalar1=0.001)
            agg_r = aggp.tile([128, ntap * C], BF16, tag=f'aggr_{bi}',
                              name='agg_r')
            if upto >= 3:
                att_bc = smalls.tile([128, KEXP], F32, tag='att_bc',
                                     name='att_bc')
                if upto >= 4:
                    diag = smalls.tile([128, KEXP], F32, tag='diag',
                                       name='diag')
                    nc.vector.tensor_scalar_mul(out=diag,
                                                in0=consts['diag_mask'],
                                                scalar1=att_s)
                    bc_ps = psum_small.tile([128, KEXP], F32, tag='sm',
                                            name='bc_ps')
                    nc.tensor.matmul(bc_ps[0:C, :],
                                     lhsT=consts['bc_mask'][0:KEXP, 0:C],
                                     rhs=diag[0:KEXP, :], start=True,
                                     stop=True)
                    nc.tensor.matmul(bc_ps[C:128, :],
                                     lhsT=consts['bc_mask'][C:C + KEXP, 0:C],
                                     rhs=diag[C:C + KEXP, :], start=True,
                                     stop=True)
                    nc.scalar.activation(out=att_bc, in_=bc_ps,
                                         func=mybir.ActivationFunctionType.Copy)
                else:
                    nc.vector.memset(att_bc, 0.25)
                if upto != 4:
                    agg = aggp.tile([128, ntap * C], F32, tag=f'agg_{bi}',
                                    name='agg')
                    nc.vector.tensor_scalar_mul(out=agg, in0=w4[:, 0],
                                                scalar1=att_bc[:, 0:1])
                    nc.vector.scalar_tensor_tensor(
                        out=agg_r, in0=w4[:, 1], scalar=att_bc[:, 1:2],
                        in1=agg, op0=mybir.AluOpType.mult,
                        op1=mybir.AluOpType.add)
                else:
                    nc.vector.tensor_copy(out=agg_r, in_=w4[:, 0])
            else:
                nc.vector.tensor_copy(out=agg_r, in_=w4[:, 0])
            it3 = it.rearrange('c (r q) -> c r q', q=pw)
            for t in range(N_TILES):
                pt = psum_conv.tile([128, NT], F32, tag='pt', name='pt')
                r0 = ROWS_PER_TILE * t
                for j, (dy, dx) in enumerate(taps):
                    st, sp = (j == 0), (j == ntap - 1)
                    nc.tensor.matmul(
                        pt[0:C, :], lhsT=agg_r[0:C, j * C:(j + 1) * C],
                        rhs=it3[0:C, r0 + dy:r0 + dy + ROWS_PER_TILE,
                                dx:dx + W],
                        start=st, stop=sp, skip_group_check=True)
                    nc.tensor.matmul(
                        pt[C:128, :], lhsT=agg_r[C:128, j * C:(j + 1) * C],
                        rhs=it3[C:128, r0 + dy:r0 + dy + ROWS_PER_TILE,
                                dx:dx + W],
                        start=st, stop=sp, skip_group_check=True)
                dst = ot[:, t * NT:(t + 1) * NT]
                nc.scalar.activation(out=dst, in_=pt,
                                     func=mybir.ActivationFunctionType.Copy)

        # ---------- schedule: branch-major with pipelined ARs ----------
        import os
        kvar = int(os.environ.get('KVAR', '3'))
        if kvar in (5, 6, 7, 8):
            # 5: pooled only; 6: +attMM/sigmoid; 7: +memset att_bc + agg ops;
            # 8: +diag/bcMM/evac, agg via copy
            upto = {5: 1, 6: 2, 7: 3, 8: 4}[kvar]
            for bi in range(3):
                conv_pair_part(bi, 0, upto)
                conv_pair_part(bi, 1, upto)
                raw_store(bi)
        elif kvar == 0:
            conv_pair(0, 0)
            raw_store(0)
        elif kvar == 4:
            for bi in range(3):
                conv_pair_noatt(bi, 0)
                conv_pair_noatt(bi, 1)
                raw_store(bi)
        elif kvar == 1:
            for bi in range(3):
                conv_pair(bi, 0)
                conv_pair(bi, 1)
                raw_store(bi)
        elif kvar == 2:
            for bi in range(3):
                conv_pair(bi, 0)
                conv_pair(bi, 1)
                branch_reduce(bi)
                raw_store(bi)
        else:
            conv_pair(0, 0)
            conv_pair(0, 1)
            branch_reduce(0)
            conv_pair(1, 0)
            conv_pair(1, 1)
            branch_reduce(1)
            branch_norm_store(0)
            conv_pair(2, 0)
            conv_pair(2, 1)
            branch_reduce(2)
            branch_norm_store(1)
            branch_norm_store(2)


_NC_CACHE = None


def _get_nc():
    global _NC_CACHE
    if _NC_CACHE is None:
        _NC_CACHE = _build_nc()
    return _NC_CACHE


def _host_weights(w, taps):
    """w [K, O, Cin, kh, kw] -> [128, K, ntap*64] f32 lhsT layout, dup halves."""
    k, o, cin, kh, kw = w.shape
    ntap = len(taps)
    wt = np.zeros((128, k, ntap * C), np.float32)
    for j, (dy, dx) in enumerate(taps):
        blk = w[:, :, :, dy if kh > 1 else 0, dx if kw > 1 else 0]
        blk = blk.transpose(0, 2, 1)  # [k, cin, o]
        wt[0:C, :, j * C:(j + 1) * C] = blk.transpose(1, 0, 2)
        wt[C:128, :, j * C:(j + 1) * C] = blk.transpose(1, 0, 2)
    return np.ascontiguousarray(wt)


def _prep_in_maps(inputs):
    x = np.ascontiguousarray(inputs['x'], dtype=np.float32)
    n_total = x.shape[0]
    pads = [(1, 1), (1, 0), (0, 1)]
    xpad = []
    for bi, (bn, (ph, pw), taps) in enumerate(BR):
        ph_, pw_ = pads[bi]
        sl = x[:, C * (bi + 1):C * (bi + 2)]  # [N, 64, H, W]
        p = np.zeros((n_total // 2, 128, ph, pw), ml_dtypes.bfloat16)
        sl2 = sl.reshape(n_total // 2, 2, C, H, W)
        p[:, 0:C, ph_:ph_ + H, pw_:pw_ + W] = sl2[:, 0]
        p[:, C:128, ph_:ph_ + H, pw_:pw_ + W] = sl2[:, 1]
        xpad.append(np.ascontiguousarray(p.reshape(n_total // 2, 128, ph * pw)))
    x0_full = np.ascontiguousarray(x[:, 0:C].reshape(n_total, C, HW))

    shared = {}
    names = [('sq', 'w_sq', 'att_w_sq', 'att_b_sq', 'g_sq', 'b_sq'),
             ('v', 'w_v', 'att_w_v', 'att_b_v', 'g_v', 'b_v'),
             ('h', 'w_h', 'att_w_h', 'att_b_h', 'g_h', 'b_h')]
    att_w_all = np.zeros((128, 3, KEXP), np.float32)
    att_b_all = np.zeros((128, 3), np.float32)
    gb_all = np.zeros((C, 2, 3), np.float32)
    for bi, (bn, wk, awk, abk, gk, bk) in enumerate(names):
        w = np.asarray(inputs[wk], dtype=np.float32)
        shared[f'w_{bn}'] = _host_weights(w, BR[bi][2])
        aw = np.asarray(inputs[awk], np.float32).T / float(HW)  # [C, K]
        att_w_all[0:C, bi, :] = aw
        att_w_all[C:128, bi, :] = aw
        ab = np.asarray(inputs[abk], np.float32)
        att_b_all[:, bi] = ab[np.arange(128) % KEXP]
        gb_all[:, 0, bi] = np.asarray(inputs[gk], np.float32)
        gb_all[:, 1, bi] = np.asarray(inputs[bk], np.float32)
    shared['att_w'] = att_w_all
    shared['att_b'] = att_b_all

    dm = np.zeros((128, KEXP), np.float32)
    for j in range(KEXP):
        dm[j, j] = 1.0
        dm[C + j, j] = 1.0
    shared['diag_mask'] = dm
    shared['bc_mask'] = np.ones((128, 128), np.float32)
    fm = np.zeros((128, C), np.float32)
    fm[np.arange(128), np.arange(128) % C] = 1.0
    shared['fold_mask'] = fm
    shared['dup_mask'] = np.ascontiguousarray(fm.T)
    shared['gb'] = gb_all

    in_maps = []
    for ci in range(N_CORES):
        m = dict(shared)
        m['x0'] = x0_full[ci * NS:(ci + 1) * NS]
        for bi, (bn, _, _) in enumerate(BR):
            m[f'xp_{bn}'] = xpad[bi][ci * NPAIR:(ci + 1) * NPAIR]
        in_maps.append(m)
    return in_maps


def run_raw(inputs, trace=False, **kwargs):
    """Build+run; returns (full_output, BassKernelResults)."""
    nc = _get_nc()
    in_maps = _prep_in_maps(inputs)
    res = bass_utils.run_bass_kernel_spmd(
        nc, in_maps, core_ids=list(range(N_CORES)), trace=trace, **kwargs)
    full = np.concatenate([res.results[i]['out'] for i in range(N_CORES)], axis=0)
    return full, res


def kernel(**inputs):
    full, _ = run_raw(inputs)
    return full
